# revision 27
# baseline (speedup 1.0000x reference)
"""BertAttention (QKV proj + MHA + output proj + residual + LayerNorm) on 8 TRN2 NeuronCores.

Sharding: batch (4-way) x query-sequence-half (2-way) => 8 shards, no collectives.
Core c handles batch b=c//2, query half c%2. Each core computes K/V for its full
batch sequence (all heads) and Q/attention/output-proj/LayerNorm for its 1024
query rows. K/V projection work is duplicated across the 2 cores sharing a batch;
in exchange there is zero cross-core communication.

The host permutes each core's X rows so its query half comes first — attention is
permutation-invariant over keys as long as (K, V, mask) share the permutation, so
the program is identical across cores (pure SPMD) with no per-core indices.

Host pre-stages inputs: X transposed to [H, S] bf16 (feature on partitions after
DMA), weights cast to bf16, residual rows kept fp32. This removes all on-device
casts and PE transposes and halves the load DMA bytes.

Layouts (SBUF partition dim first):
  xt:      [128, H/128, S]   transposed activations, bf16 (direct DMA)
  Kt:      [128, H/128, S]   transposed keys (feature on partitions), bf16
  Qt:      [128, H/128, SH]  transposed, bf16
  V:       [128, S/128, NH*65] natural ([tok, head-dim]) with a ones column per
           head at slot 64 — the PV matmul then yields sum(exp) as row 64 for free
  scores:  St[ktok, qtok] in PSUM; softmax sum over ktok (the partition dim) comes
           from the ones-column trick; max-subtraction safely skipped (|s| <~ 1)
  ctx:     [128, NH/2, SH]   transposed (head dim on partitions), bf16
  out:     natural [qtok, H] — residual add + LayerNorm along the free dim.

Schedule: K/Q tile 0 + V chunk 0 projected upfront (~25us, overlapped with the
weight DMA), then the 16-head attention loop starts; remaining projection groups
are deadline-paced into the loop (one group per scores iteration) as PE
gap-filler. PV PSUM accumulators ping-pong across heads so the softmax-sum
eviction (DMA roundtrip broadcast + reciprocal + multiply) never stalls the PE.
"""

from contextlib import ExitStack

import numpy as np
import ml_dtypes

import bass_rust
import concourse.bass as bass
import concourse.mybir as mybir
from concourse.tile import TileContext
from concourse.bass_utils import run_bass_kernel_spmd

FP = mybir.dt.float32
BF = mybir.dt.bfloat16
E4 = mybir.dt.float8e4
DR = mybir.MatmulPerfMode.DoubleRow
AF = mybir.ActivationFunctionType
OP = mybir.AluOpType

N_CORES = 8
EPS = 1e-12

# The walrus build in this toolchain rejects instructions that carry more than
# one sync-wait command ("Too many sync wait commands", CoreV2/V3 setupSyncWait),
# while Tile freely attaches several semaphore waits to one instruction (and the
# TileContext exit drain aggregates one wait per logical processor). Hoist the
# excess waits onto standalone InstEventSemaphore carriers on the same engine,
# placed immediately before the instruction — engine streams are serial, so the
# gating semantics are identical.
_MAX_WAITS_PER_INST = 1


def _split_sync_waits(nc, cap=_MAX_WAITS_PER_INST):
    n_split = 0
    for fn in nc.m.functions:
        for bb in fn.blocks:
            insts = list(bb.instructions)
            out = []
            changed = False
            for ins in insts:
                si = ins.sync_info
                waits = list(si.on_wait) if (si is not None and si.on_wait) else []
                if len(waits) > cap:
                    head, tail = waits[: len(waits) - cap], waits[len(waits) - cap :]
                    for j, w in enumerate(head):
                        ev = mybir.InstEventSemaphore(
                            name=f"{ins.name}-sw{j}",
                            engine=ins.engine,
                            ins=[],
                            outs=[],
                            sync_info=bass_rust.SyncInfo(on_wait=[w], on_update=[]),
                        )
                        out.append(ev)
                        n_split += 1
                    si.on_wait = tail
                    changed = True
                out.append(ins)
            if changed:
                bb.instructions[:] = out
    return n_split


def _dram_row_bcast(handle, p, n):
    """AP reading DRAM vector [n] broadcast across p partitions."""
    return bass.AP(tensor=handle, offset=0, ap=[[0, p], [1, n]])


def _build(s, h, nh, sh, flags):
    """Build the per-core Bass program. flags: which bias/affine inputs matter."""
    hd = h // nh
    assert hd == 64, "head packing assumes head_dim 64 (2 heads per 128 partitions)"
    kt_n = h // 128  # contraction tiles over hidden dim
    tt_n = s // 128  # key-token tiles
    qt_n = sh // 128  # query-token tiles
    scale = 1.0 / float(np.sqrt(hd))

    nc = bass.Bass(target_bir_lowering=False)
    x = nc.dram_tensor("x", [sh, h], FP, kind="ExternalInput")  # residual rows
    xt_d = nc.dram_tensor("xt", [h, s], E4, kind="ExternalInput")  # transposed
    mask = nc.dram_tensor("mask", [s], FP, kind="ExternalInput")
    w_dram = {
        n: nc.dram_tensor(n, [h, h], E4, kind="ExternalInput")
        for n in ("wq", "wk", "wv", "wo")
    }
    vec_dram = {
        n: nc.dram_tensor(n, [h], FP, kind="ExternalInput")
        for n in ("bq", "bk", "bv", "bo", "ln_gamma", "ln_beta")
        if flags[n]
    }
    out = nc.dram_tensor("out", [sh, h], FP, kind="ExternalOutput")

    with TileContext(nc) as tc, ExitStack() as st_all:
        persist = st_all.enter_context(tc.tile_pool(name="persist", bufs=1))
        dram = st_all.enter_context(tc.tile_pool(name="dram", bufs=1, space="DRAM"))
        qt = persist.tile([128, kt_n, sh], BF)
        kt = persist.tile([128, kt_n, s], BF)
        vsb = persist.tile([128, tt_n, nh * 65], E4)
        ctx_t = persist.tile([128, nh // 2, sh], E4)
        mask_sb = persist.tile([128, tt_n], FP)
        eps_sb = persist.tile([128, 1], FP)

        nc.vector.memset(eps_sb, EPS)
        nc.sync.dma_start(out=mask_sb, in_=mask[:].rearrange("(t p) -> p t", p=128))

        # Schraudolph fast-exp (exp(y) ~ bitcast_f32(i32(y*A + B))) constants;
        # a few exp tiles per head run as DVE int-convert + GpSimd bitcast-copy
        # to offload the Scalar engine (max ~4% per-element error, harmless
        # after softmax normalization and the residual).
        FE_A = 12102203.161561485
        FE_B = 1064866805.0
        fe_bcol = persist.tile([128, tt_n], FP, name="fe_bcol")
        nc.vector.tensor_scalar(
            out=fe_bcol,
            in0=mask_sb,
            scalar1=FE_A,
            scalar2=FE_B,
            op0=OP.mult,
            op1=OP.add,
        )

        # bias columns for Qt/Kt evictions (partition = output feature in tile)
        bias_cols = {}
        for name in ("bq", "bk"):
            if flags[name]:
                col = persist.tile([128, kt_n], FP, name=f"{name}_col")
                nc.sync.dma_start(
                    out=col, in_=vec_dram[name][:].rearrange("(t p) -> p t", p=128)
                )
                bias_cols[name] = col
        # rows broadcast across partitions for V/out bias and LN affine
        bcast = {}
        for name in ("bv", "bo", "ln_gamma", "ln_beta"):
            if flags[name]:
                t = persist.tile([128, h], FP, name=f"{name}_bc")
                nc.sync.dma_start(out=t, in_=_dram_row_bcast(vec_dram[name], 128, h))
                bcast[name] = t

        # ones columns in V (slot 64 of each 65-wide head block)
        for m in range(tt_n):
            v_view = vsb[:, m, :].rearrange("p (a e) -> p a e", e=65)
            nc.vector.memset(v_view[:, :, 64:65], 1.0)

        with ExitStack() as st_proj:
            xtpool = st_proj.enter_context(tc.tile_pool(name="xtpool", bufs=1))
            wbuf = st_proj.enter_context(tc.tile_pool(name="wbuf", bufs=3))

            xt = xtpool.tile([128, kt_n, s], E4)

            # slot assignment: wv takes slot 0 so the late wo load (issued
            # after the last V fill group) can reuse it; wk/wq live through
            # the whole attention loop (K/Q fill groups read them).
            wv_bf = wbuf.tile([128, kt_n, h], E4, name="wv_bf", tag="w")
            wk_bf = wbuf.tile([128, kt_n, h], E4, name="wk_bf", tag="w")
            wq_bf = wbuf.tile([128, kt_n, h], E4, name="wq_bf", tag="w")

            def load_w(dname, dst):
                for k in range(kt_n):
                    nc.sync.dma_start(
                        out=dst[:, k, :], in_=w_dram[dname][k * 128 : (k + 1) * 128, :]
                    )

            # DMA issue order = priority: xt + wk first (unblocks K tile 0),
            # then wq, then wv. wo is issued much later.
            for k in range(kt_n):
                nc.sync.dma_start(
                    out=xt[:, k, :], in_=xt_d[k * 128 : (k + 1) * 128, :]
                )
            load_w("wk", wk_bf)
            load_w("wq", wq_bf)
            load_w("wv", wv_bf)

            def kq_group(w_bf, dst, bias_col, m, n0, pool, pool_shape, tag):
                """One K/Q projection PSUM group: 8 accumulating matmuls + evict."""
                n1 = min(n0 + 512, dst.shape[2])
                ps = pool.tile(pool_shape, FP, name="projp", tag=tag)
                for k in range(0, kt_n, 2):
                    nc.tensor.matmul(
                        ps[:, : n1 - n0],
                        w_bf[:, k : k + 2, m * 128 : (m + 1) * 128],
                        xt[:, k : k + 2, n0:n1],
                        start=(k == 0),
                        stop=(k == kt_n - 2),
                        perf_mode=DR,
                    )
                if bias_col is not None:
                    nc.vector.tensor_scalar_add(
                        out=dst[:, m, n0:n1],
                        in0=ps[:, : n1 - n0],
                        scalar1=bias_col[:, m : m + 1],
                    )
                else:
                    nc.vector.tensor_copy(out=dst[:, m, n0:n1], in_=ps[:, : n1 - n0])

            def v_group(m, n0, pool, pool_shape, tag):
                ps = pool.tile(pool_shape, FP, name="projp", tag=tag)
                for k in range(0, kt_n, 2):
                    nc.tensor.matmul(
                        ps[:, :512],
                        xt[:, k : k + 2, m * 128 : (m + 1) * 128],
                        wv_bf[:, k : k + 2, n0 : n0 + 512],
                        start=(k == 0),
                        stop=(k == kt_n - 2),
                        perf_mode=DR,
                    )
                dst = vsb[:, m, :].rearrange("p (a e) -> p a e", e=65)[
                    :, n0 // 64 : n0 // 64 + 8, 0:64
                ]
                src = ps[:, :512].rearrange("p (a e) -> p a e", e=64)
                if "bv" in bcast:
                    nc.vector.tensor_add(
                        out=dst,
                        in0=src,
                        in1=bcast["bv"][:, n0 : n0 + 512].rearrange(
                            "p (a e) -> p a e", e=64
                        ),
                    )
                else:
                    nc.vector.tensor_copy(out=dst, in_=src)

            # ---- upfront projections (overlap the weight DMA) ----
            with tc.tile_pool(name="projps", bufs=2, space="PSUM") as projps:
                for n0 in range(0, s, 512):
                    kq_group(
                        wk_bf, kt, bias_cols.get("bk"), 0, n0, projps, [128, 512], "projp"
                    )
                for n0 in range(0, sh, 512):
                    kq_group(
                        wq_bf, qt, bias_cols.get("bq"), 0, n0, projps, [128, 512], "projp"
                    )
                for m in range(tt_n):
                    v_group(m, 0, projps, [128, 512], "projp")

            # fill tasks, deadline in scores-iteration units (16 per head)
            tasks = []
            for m in range(1, kt_n):
                for n0 in range(0, s, 512):
                    tasks.append((32 * m, "k", m, n0))
                for n0 in range(0, sh, 512):
                    tasks.append((32 * m, "q", m, n0))
            for m in range(tt_n):
                tasks.append((8 * tt_n + m, "v", m, 512))
            tasks.sort()
            n_it_total = nh * tt_n
            tasks = [
                (min(dl - 8, round((i + 0.5) * n_it_total / len(tasks))), kind, fm, fn0)
                for i, (dl, kind, fm, fn0) in enumerate(tasks)
            ]
            tasks.sort()
            wo_issued = False
            n_v_left = tt_n

            # ---- attention, with projection fill interleaved ----
            with ExitStack() as st_att:
                psb = st_att.enter_context(tc.tile_pool(name="psb", bufs=3))
                rpool = st_att.enter_context(tc.tile_pool(name="rpool", bufs=2))
                stps = st_att.enter_context(
                    tc.tile_pool(name="stps", bufs=3, space="PSUM")
                )
                pvps = st_att.enter_context(
                    tc.tile_pool(name="pvps", bufs=1, space="PSUM")
                )
                LOOKAHEAD = 24

                def run_task(kind, fm, fn0):
                    nonlocal n_v_left, wo_issued
                    if kind == "k":
                        kq_group(
                            wk_bf, kt, bias_cols.get("bk"), fm, fn0, stps, [128, sh], "stp"
                        )
                    elif kind == "q":
                        kq_group(
                            wq_bf, qt, bias_cols.get("bq"), fm, fn0, stps, [128, sh], "stp"
                        )
                    else:
                        v_group(fm, fn0, stps, [128, sh], "stp")
                        n_v_left -= 1
                        if n_v_left == 0 and not wo_issued:
                            wo_issued = True
                            wo_tiles.append(wbuf.tile([128, kt_n, h], E4, name="wo_bf", tag="w"))
                            load_w("wo", wo_tiles[0])

                wo_tiles = []
                it = 0
                for hh in range(nh):
                    mt, po = hh // 2, 64 * (hh % 2)
                    pv = pvps.tile([65, sh], FP, name="pvp")
                    for m in range(tt_n):
                        while tasks and tasks[0][0] <= it:
                            _, kind, fm, fn0 = tasks.pop(0)
                            run_task(kind, fm, fn0)
                        if tasks and tasks[0][0] <= it + LOOKAHEAD:
                            _, kind, fm, fn0 = tasks.pop(0)
                            run_task(kind, fm, fn0)
                        stt = stps.tile([128, sh], FP, name="stp", tag="stp")
                        for c in range(0, sh, 512):
                            nc.tensor.matmul(
                                stt[:, c : c + 512],
                                kt[po : po + 64, mt, m * 128 : (m + 1) * 128],
                                qt[po : po + 64, mt, c : c + 512],
                                start=True,
                                stop=True,
                            )
                        if m % 2 == 0:
                            p_pair = psb.tile([128, 2, sh], E4, name="pexp")
                        if m in (3, 8, 13):
                            fei = rpool.tile(
                                [128, sh], mybir.dt.int32, name="fei", bufs=3
                            )
                            nc.vector.tensor_scalar(
                                out=fei,
                                in0=stt,
                                scalar1=FE_A * scale / 256.0,
                                scalar2=fe_bcol[:, m : m + 1],
                                op0=OP.mult,
                                op1=OP.add,
                            )
                            nc.vector.tensor_copy(
                                out=p_pair[:, m % 2, :], in_=fei[:, :].bitcast(FP)
                            )
                        else:
                            nc.scalar.activation(
                                p_pair[:, m % 2, :],
                                stt,
                                AF.Exp,
                                bias=mask_sb[:, m : m + 1],
                                scale=scale / 256.0,
                            )
                        if m % 2 == 1:
                            for c in range(0, sh, 512):
                                nc.tensor.matmul(
                                    pv[:, c : c + 512],
                                    vsb[:, m - 1 : m + 1, hh * 65 : (hh + 1) * 65],
                                    p_pair[:, 0:2, c : c + 512],
                                    start=(m == 1),
                                    stop=(m == tt_n - 1),
                                    perf_mode=DR,
                                )
                        it += 1
                    # quick-free eviction: one DVE copy releases the single PV
                    # bank; the sum-row DRAM-roundtrip broadcast, reciprocal and
                    # normalize all run from the SBUF copy, off the PE path.
                    pvc = rpool.tile([65, sh], FP, name="pvc", bufs=3)
                    nc.scalar.copy(out=pvc, in_=pv)
                    r_dram = dram.tile([sh], FP, name="rdram", tag="rdram", bufs=3)
                    nc.sync.dma_start(out=r_dram, in_=pvc[64:65, :])
                    # reciprocal cost scales with free size only: fold the 1024
                    # sums to [128, 8] for the reciprocal, then roundtrip again
                    # to broadcast the results across 64 partitions.
                    rbt = rpool.tile([128, sh // 128], FP, name="rbt", bufs=3)
                    nc.sync.dma_start(
                        out=rbt, in_=r_dram[:].rearrange("(p t) -> p t", p=128)
                    )
                    rit = rpool.tile([128, sh // 128], FP, name="rit", bufs=3)
                    nc.vector.reciprocal(rit, rbt)
                    r2_dram = dram.tile([sh], FP, name="r2dram", tag="r2dram", bufs=3)
                    nc.sync.dma_start(
                        out=r2_dram[:].rearrange("(p t) -> p t", p=128), in_=rit
                    )
                    rinv = rpool.tile([64, sh], FP, name="rinv", bufs=3)
                    nc.sync.dma_start(
                        out=rinv,
                        in_=bass.AP(
                            tensor=r2_dram.tensor,
                            offset=r2_dram.offset,
                            ap=[[0, 64], [1, sh]],
                        ),
                    )
                    nc.vector.tensor_mul(
                        out=ctx_t[po : po + 64, mt, :],
                        in0=pvc[0:64, :],
                        in1=rinv,
                    )
                for _, kind, fm, fn0 in tasks:  # leftovers (shouldn't happen)
                    run_task(kind, fm, fn0)
            wo_bf = wo_tiles[0]

            # ---- output projection + residual + LayerNorm (natural layout) ----
            with (
                tc.tile_pool(name="ops", bufs=8, space="PSUM") as ops,
                tc.tile_pool(name="xrp", bufs=qt_n) as xrp,
                tc.tile_pool(name="osb", bufs=3) as osb,
                tc.tile_pool(name="lnp", bufs=4) as lnp,
            ):
                xres_tiles = []
                for m in range(qt_n):
                    xr = xrp.tile([128, h], FP, name="xres", tag="xres")
                    nc.sync.dma_start(out=xr, in_=x[m * 128 : (m + 1) * 128, :])
                    xres_tiles.append(xr)
                for m in range(qt_n):
                    pss = []
                    for n0 in range(0, h, 512):
                        ps = ops.tile([128, 512], FP, name="op")
                        # ctx_t tile mt holds heads 2mt / 2mt+1 on partitions
                        # 0-63 / 64-127, exactly matching Wo rows mt*128..(mt+1)*128,
                        # so one K=128 matmul contracts both heads at once.
                        for mt in range(0, nh // 2, 2):
                            nc.tensor.matmul(
                                ps,
                                ctx_t[:, mt : mt + 2, m * 128 : (m + 1) * 128],
                                wo_bf[:, mt : mt + 2, n0 : n0 + 512],
                                start=(mt == 0),
                                stop=(mt == nh // 2 - 2),
                                perf_mode=DR,
                            )
                        pss.append((n0, ps))
                    xres = xres_tiles[m]
                    o = osb.tile([128, h], FP, name="osum")
                    for n0, ps in pss:
                        nc.vector.tensor_add(
                            out=o[:, n0 : n0 + 512], in0=ps, in1=xres[:, n0 : n0 + 512]
                        )
                    if "bo" in bcast:
                        nc.vector.tensor_add(out=o, in0=o, in1=bcast["bo"])
                    nsub = (h + 511) // 512
                    stats = lnp.tile([128, nsub, 6], FP, name="stats")
                    for i in range(nsub):
                        nc.vector.bn_stats(
                            out=stats[:, i, :], in_=o[:, i * 512 : (i + 1) * 512]
                        )
                    mv = lnp.tile([128, 2], FP, name="mv")
                    nc.vector.bn_aggr(out=mv, in_=stats)
                    std = lnp.tile([128, 1], FP, name="std")
                    nc.scalar.activation(std, mv[:, 1:2], AF.Sqrt, bias=eps_sb)
                    inv = lnp.tile([128, 1], FP, name="inv")
                    nc.vector.reciprocal(inv, std)
                    nb = lnp.tile([128, 1], FP, name="nb")
                    nc.vector.tensor_tensor(
                        out=nb, in0=mv[:, 0:1], in1=inv, op=OP.mult
                    )
                    nc.vector.tensor_scalar_mul(out=nb, in0=nb, scalar1=-1.0)
                    y = osb.tile([128, h], FP, name="yout")
                    nc.scalar.activation(
                        out=y, in_=o, func=AF.Identity, bias=nb, scale=inv
                    )
                    if "ln_gamma" in bcast:
                        nc.vector.tensor_mul(out=y, in0=y, in1=bcast["ln_gamma"])
                    if "ln_beta" in bcast:
                        nc.vector.tensor_add(out=y, in0=y, in1=bcast["ln_beta"])
                    nc.sync.dma_start(out=out[m * 128 : (m + 1) * 128, :], in_=y)

    _split_sync_waits(nc)
    return nc


_NC_CACHE = {}


def _get_nc(s, h, nh, sh, flags):
    key = (s, h, nh, sh, tuple(sorted(flags.items())))
    if key not in _NC_CACHE:
        _NC_CACHE[key] = _build(s, h, nh, sh, flags)
    return _NC_CACHE[key]


def _prepare(hidden_states, attention_mask, Wq, bq, Wk, bk, Wv, bv, Wo, bo, ln_gamma, ln_beta):
    hs = np.ascontiguousarray(np.asarray(hidden_states, dtype=np.float32))
    b_, s_, h_ = hs.shape
    nh_ = h_ // 64
    sh_ = s_ // 2
    am = np.asarray(attention_mask, dtype=np.float32).reshape(b_, s_)
    flags = {
        "bq": bool(np.any(np.asarray(bq))),
        "bk": bool(np.any(np.asarray(bk))),
        "bv": bool(np.any(np.asarray(bv))),
        "bo": bool(np.any(np.asarray(bo))),
        "ln_gamma": not bool(np.all(np.asarray(ln_gamma) == 1.0)),
        "ln_beta": bool(np.any(np.asarray(ln_beta))),
    }
    nc = _get_nc(s_, h_, nh_, sh_, flags)

    f32c = lambda a: np.ascontiguousarray(np.asarray(a, dtype=np.float32))
    f8c = lambda a, sc: np.ascontiguousarray(
        (np.asarray(a, dtype=np.float32) * sc).astype(ml_dtypes.float8_e4m3fn)
    )
    # weights x16 in fp8 (keeps small values out of the subnormal range);
    # K/Q both carry x16 so scores carry x256, folded into the Exp scale.
    # ctx_t carries x64 (x16 from V, x4 from the sum eviction), Wo x16, so
    # the out-proj PSUM carries x1024 — matched by scaling the residual
    # x1024 on the host. LayerNorm is scale-invariant, so the output is
    # unchanged.
    shared = {
        "wq": f8c(Wq, 16.0),
        "wk": f8c(Wk, 16.0),
        "wv": f8c(Wv, 64.0),
        "wo": f8c(Wo, 16.0),
    }
    scales = {"bq": 16.0, "bk": 16.0, "bv": 64.0, "bo": 1024.0}
    for name, arr in (
        ("bq", bq),
        ("bk", bk),
        ("bv", bv),
        ("bo", bo),
        ("ln_gamma", ln_gamma),
        ("ln_beta", ln_beta),
    ):
        if flags[name]:
            shared[name] = f32c(np.asarray(arr) * scales.get(name, 1.0))

    in_maps = []
    for c in range(N_CORES):
        bb, half = c // 2, c % 2
        mine = slice(half * sh_, (half + 1) * sh_)
        other = slice((1 - half) * sh_, (2 - half) * sh_)
        xp = np.concatenate([hs[bb, mine], hs[bb, other]], axis=0)
        xt = np.ascontiguousarray(xp.T.astype(ml_dtypes.float8_e4m3fn))
        mp = np.ascontiguousarray(np.concatenate([am[bb, mine], am[bb, other]]))
        in_maps.append(
            {
                "x": np.ascontiguousarray(xp[:sh_] * 1024.0),
                "xt": xt,
                "mask": mp,
                **shared,
            }
        )
    return nc, in_maps, (b_, s_, h_, sh_)


def _assemble(results, shape):
    b_, s_, h_, sh_ = shape
    out = np.empty((b_, s_, h_), dtype=np.float32)
    for c in range(N_CORES):
        bb, half = c // 2, c % 2
        out[bb, half * sh_ : (half + 1) * sh_] = results[c]["out"]
    return out


def kernel(**inputs) -> np.ndarray:
    nc, in_maps, shape = _prepare(**inputs)
    res = run_bass_kernel_spmd(nc, in_maps, core_ids=list(range(N_CORES)))
    return _assemble(res.results, shape)


# revision 28
# speedup vs baseline: 1.1469x; 1.1469x over previous
"""BertAttention (QKV proj + MHA + output proj + residual + LayerNorm) on 8 TRN2 NeuronCores.

Sharding: batch (4-way) x query-sequence-half (2-way) => 8 shards, no collectives.
Core c handles batch b=c//2, query half c%2. Each core computes K/V for its full
batch sequence (all heads) and Q/attention/output-proj/LayerNorm for its 1024
query rows. K/V projection work is duplicated across the 2 cores sharing a batch;
in exchange there is zero cross-core communication.

The host permutes each core's X rows so its query half comes first — attention is
permutation-invariant over keys as long as (K, V, mask) share the permutation, so
the program is identical across cores (pure SPMD) with no per-core indices.

Host pre-stages inputs: X transposed to [H, S] bf16 (feature on partitions after
DMA), weights cast to bf16, residual rows kept fp32. This removes all on-device
casts and PE transposes and halves the load DMA bytes.

Layouts (SBUF partition dim first):
  xt:      [128, H/128, S]   transposed activations, bf16 (direct DMA)
  Kt:      [128, H/128, S]   transposed keys (feature on partitions), bf16
  Qt:      [128, H/128, SH]  transposed, bf16
  V:       [128, S/128, NH*65] natural ([tok, head-dim]) with a ones column per
           head at slot 64 — the PV matmul then yields sum(exp) as row 64 for free
  scores:  St[ktok, qtok] in PSUM; softmax sum over ktok (the partition dim) comes
           from the ones-column trick; max-subtraction safely skipped (|s| <~ 1)
  ctx:     [128, NH/2, SH]   transposed (head dim on partitions), bf16
  out:     natural [qtok, H] — residual add + LayerNorm along the free dim.

Schedule: K/Q tile 0 + V chunk 0 projected upfront (~25us, overlapped with the
weight DMA), then the 16-head attention loop starts; remaining projection groups
are deadline-paced into the loop (one group per scores iteration) as PE
gap-filler. PV PSUM accumulators ping-pong across heads so the softmax-sum
eviction (DMA roundtrip broadcast + reciprocal + multiply) never stalls the PE.
"""

from contextlib import ExitStack

import numpy as np
import ml_dtypes

import bass_rust
import concourse.bass as bass
import concourse.mybir as mybir
from concourse.tile import TileContext
from concourse.bass_utils import run_bass_kernel_spmd

FP = mybir.dt.float32
BF = mybir.dt.bfloat16
E4 = mybir.dt.float8e4
DR = mybir.MatmulPerfMode.DoubleRow
AF = mybir.ActivationFunctionType
OP = mybir.AluOpType

N_CORES = 8
EPS = 1e-12

# The walrus build in this toolchain rejects instructions that carry more than
# one sync-wait command ("Too many sync wait commands", CoreV2/V3 setupSyncWait),
# while Tile freely attaches several semaphore waits to one instruction (and the
# TileContext exit drain aggregates one wait per logical processor). Hoist the
# excess waits onto standalone InstEventSemaphore carriers on the same engine,
# placed immediately before the instruction — engine streams are serial, so the
# gating semantics are identical.
_MAX_WAITS_PER_INST = 1


def _split_sync_waits(nc, cap=_MAX_WAITS_PER_INST):
    n_split = 0
    for fn in nc.m.functions:
        for bb in fn.blocks:
            insts = list(bb.instructions)
            out = []
            changed = False
            for ins in insts:
                si = ins.sync_info
                waits = list(si.on_wait) if (si is not None and si.on_wait) else []
                if len(waits) > cap:
                    head, tail = waits[: len(waits) - cap], waits[len(waits) - cap :]
                    for j, w in enumerate(head):
                        ev = mybir.InstEventSemaphore(
                            name=f"{ins.name}-sw{j}",
                            engine=ins.engine,
                            ins=[],
                            outs=[],
                            sync_info=bass_rust.SyncInfo(on_wait=[w], on_update=[]),
                        )
                        out.append(ev)
                        n_split += 1
                    si.on_wait = tail
                    changed = True
                out.append(ins)
            if changed:
                bb.instructions[:] = out
    return n_split


def _dram_row_bcast(handle, p, n):
    """AP reading DRAM vector [n] broadcast across p partitions."""
    return bass.AP(tensor=handle, offset=0, ap=[[0, p], [1, n]])


def _build(s, h, nh, sh, flags):
    """Build the per-core Bass program. flags: which bias/affine inputs matter."""
    hd = h // nh
    assert hd == 64, "head packing assumes head_dim 64 (2 heads per 128 partitions)"
    kt_n = h // 128  # contraction tiles over hidden dim
    tt_n = s // 128  # key-token tiles
    qt_n = sh // 128  # query-token tiles
    scale = 1.0 / float(np.sqrt(hd))

    nc = bass.Bass(target_bir_lowering=False)
    x = nc.dram_tensor("x", [sh, h], FP, kind="ExternalInput")  # residual rows
    xt_d = nc.dram_tensor("xt", [h, s], E4, kind="ExternalInput")  # transposed
    mask = nc.dram_tensor("mask", [s], FP, kind="ExternalInput")
    w_dram = {
        n: nc.dram_tensor(n, [h, h], E4, kind="ExternalInput")
        for n in ("wq", "wk", "wv", "wo")
    }
    vec_dram = {
        n: nc.dram_tensor(n, [h], FP, kind="ExternalInput")
        for n in ("bq", "bk", "bv", "bo", "ln_gamma", "ln_beta")
        if flags[n]
    }
    out = nc.dram_tensor("out", [sh, h], FP, kind="ExternalOutput")

    with TileContext(nc) as tc, ExitStack() as st_all:
        persist = st_all.enter_context(tc.tile_pool(name="persist", bufs=1))
        dram = st_all.enter_context(tc.tile_pool(name="dram", bufs=1, space="DRAM"))
        qt = persist.tile([128, kt_n, sh], BF)
        kt = persist.tile([128, kt_n, s], BF)
        vsb = persist.tile([128, tt_n, nh * 65], E4)
        ctx_t = persist.tile([128, nh // 2, sh], E4)
        mask_sb = persist.tile([128, tt_n], FP)
        eps_sb = persist.tile([128, 1], FP)

        nc.vector.memset(eps_sb, EPS)
        nc.sync.dma_start(out=mask_sb, in_=mask[:].rearrange("(t p) -> p t", p=128))

        # Schraudolph fast-exp (exp(y) ~ bitcast_f32(i32(y*A + B))) constants;
        # a few exp tiles per head run as DVE int-convert + GpSimd bitcast-copy
        # to offload the Scalar engine (max ~4% per-element error, harmless
        # after softmax normalization and the residual).
        FE_A = 12102203.161561485
        FE_B = 1064866805.0
        fe_bcol = persist.tile([128, tt_n], FP, name="fe_bcol")
        nc.vector.tensor_scalar(
            out=fe_bcol,
            in0=mask_sb,
            scalar1=FE_A,
            scalar2=FE_B,
            op0=OP.mult,
            op1=OP.add,
        )

        # bias columns for Qt/Kt evictions (partition = output feature in tile)
        bias_cols = {}
        for name in ("bq", "bk"):
            if flags[name]:
                col = persist.tile([128, kt_n], FP, name=f"{name}_col")
                nc.sync.dma_start(
                    out=col, in_=vec_dram[name][:].rearrange("(t p) -> p t", p=128)
                )
                bias_cols[name] = col
        # rows broadcast across partitions for V/out bias and LN affine
        bcast = {}
        for name in ("bv", "bo", "ln_gamma", "ln_beta"):
            if flags[name]:
                t = persist.tile([128, h], FP, name=f"{name}_bc")
                nc.sync.dma_start(out=t, in_=_dram_row_bcast(vec_dram[name], 128, h))
                bcast[name] = t

        # ones columns in V (slot 64 of each 65-wide head block)
        for m in range(tt_n):
            v_view = vsb[:, m, :].rearrange("p (a e) -> p a e", e=65)
            nc.vector.memset(v_view[:, :, 64:65], 1.0)

        with ExitStack() as st_proj:
            xtpool = st_proj.enter_context(tc.tile_pool(name="xtpool", bufs=1))
            wbuf = st_proj.enter_context(tc.tile_pool(name="wbuf", bufs=3))

            xt = xtpool.tile([128, kt_n, s], E4)

            # slot assignment: wv takes slot 0 so the late wo load (issued
            # after the last V fill group) can reuse it; wk/wq live through
            # the whole attention loop (K/Q fill groups read them).
            wv_bf = wbuf.tile([128, kt_n, h], E4, name="wv_bf", tag="w")
            wk_bf = wbuf.tile([128, kt_n, h], E4, name="wk_bf", tag="w")
            wq_bf = wbuf.tile([128, kt_n, h], E4, name="wq_bf", tag="w")

            def load_w(dname, dst):
                for k in range(kt_n):
                    nc.sync.dma_start(
                        out=dst[:, k, :], in_=w_dram[dname][k * 128 : (k + 1) * 128, :]
                    )

            # DMA issue order = priority: xt + wk first (unblocks K tile 0),
            # then wq, then wv. wo is issued much later.
            for k in range(kt_n):
                nc.sync.dma_start(
                    out=xt[:, k, :], in_=xt_d[k * 128 : (k + 1) * 128, :]
                )
            load_w("wk", wk_bf)
            load_w("wq", wq_bf)
            load_w("wv", wv_bf)

            def kq_group(w_bf, dst, bias_col, m, n0, pool, pool_shape, tag):
                """One K/Q projection PSUM group: 8 accumulating matmuls + evict."""
                n1 = min(n0 + 512, dst.shape[2])
                ps = pool.tile(pool_shape, FP, name="projp", tag=tag)
                for k in range(0, kt_n, 2):
                    nc.tensor.matmul(
                        ps[:, : n1 - n0],
                        w_bf[:, k : k + 2, m * 128 : (m + 1) * 128],
                        xt[:, k : k + 2, n0:n1],
                        start=(k == 0),
                        stop=(k == kt_n - 2),
                        perf_mode=DR,
                    )
                if bias_col is not None:
                    nc.vector.tensor_scalar_add(
                        out=dst[:, m, n0:n1],
                        in0=ps[:, : n1 - n0],
                        scalar1=bias_col[:, m : m + 1],
                    )
                else:
                    nc.vector.tensor_copy(out=dst[:, m, n0:n1], in_=ps[:, : n1 - n0])

            def v_group(m, n0, pool, pool_shape, tag):
                ps = pool.tile(pool_shape, FP, name="projp", tag=tag)
                for k in range(0, kt_n, 2):
                    nc.tensor.matmul(
                        ps[:, :512],
                        xt[:, k : k + 2, m * 128 : (m + 1) * 128],
                        wv_bf[:, k : k + 2, n0 : n0 + 512],
                        start=(k == 0),
                        stop=(k == kt_n - 2),
                        perf_mode=DR,
                    )
                dst = vsb[:, m, :].rearrange("p (a e) -> p a e", e=65)[
                    :, n0 // 64 : n0 // 64 + 8, 0:64
                ]
                src = ps[:, :512].rearrange("p (a e) -> p a e", e=64)
                if "bv" in bcast:
                    nc.vector.tensor_add(
                        out=dst,
                        in0=src,
                        in1=bcast["bv"][:, n0 : n0 + 512].rearrange(
                            "p (a e) -> p a e", e=64
                        ),
                    )
                else:
                    nc.vector.tensor_copy(out=dst, in_=src)

            # ---- upfront projections (overlap the weight DMA) ----
            with tc.tile_pool(name="projps", bufs=2, space="PSUM") as projps:
                for n0 in range(0, s, 512):
                    kq_group(
                        wk_bf, kt, bias_cols.get("bk"), 0, n0, projps, [128, 512], "projp"
                    )
                for n0 in range(0, sh, 512):
                    kq_group(
                        wq_bf, qt, bias_cols.get("bq"), 0, n0, projps, [128, 512], "projp"
                    )
                for m in range(tt_n):
                    v_group(m, 0, projps, [128, 512], "projp")

            # fill tasks, deadline in scores-iteration units (16 per head)
            tasks = []
            for m in range(1, kt_n):
                for n0 in range(0, s, 512):
                    tasks.append((32 * m, "k", m, n0))
                for n0 in range(0, sh, 512):
                    tasks.append((32 * m, "q", m, n0))
            for m in range(tt_n):
                tasks.append((8 * tt_n + m, "v", m, 512))
            tasks.sort()
            n_it_total = nh * tt_n
            tasks = [
                (min(dl - 8, round((i + 0.5) * n_it_total / len(tasks))), kind, fm, fn0)
                for i, (dl, kind, fm, fn0) in enumerate(tasks)
            ]
            tasks.sort()
            wo_issued = False
            n_v_left = tt_n

            # ---- attention, with projection fill interleaved ----
            with ExitStack() as st_att:
                psb = st_att.enter_context(tc.tile_pool(name="psb", bufs=3))
                rpool = st_att.enter_context(tc.tile_pool(name="rpool", bufs=2))
                stps = st_att.enter_context(
                    tc.tile_pool(name="stps", bufs=3, space="PSUM")
                )
                pvps = st_att.enter_context(
                    tc.tile_pool(name="pvps", bufs=1, space="PSUM")
                )
                LOOKAHEAD = 24

                def run_task(kind, fm, fn0):
                    nonlocal n_v_left, wo_issued
                    if kind == "k":
                        kq_group(
                            wk_bf, kt, bias_cols.get("bk"), fm, fn0, stps, [128, sh], "stp"
                        )
                    elif kind == "q":
                        kq_group(
                            wq_bf, qt, bias_cols.get("bq"), fm, fn0, stps, [128, sh], "stp"
                        )
                    else:
                        v_group(fm, fn0, stps, [128, sh], "stp")
                        n_v_left -= 1
                        if n_v_left == 0 and not wo_issued:
                            wo_issued = True
                            wo_tiles.append(wbuf.tile([128, kt_n, h], E4, name="wo_bf", tag="w"))
                            load_w("wo", wo_tiles[0])

                wo_tiles = []
                it = 0
                for hh in range(nh):
                    mt, po = hh // 2, 64 * (hh % 2)
                    pv = pvps.tile([65, sh], FP, name="pvp")
                    for m in range(tt_n):
                        while tasks and tasks[0][0] <= it:
                            _, kind, fm, fn0 = tasks.pop(0)
                            run_task(kind, fm, fn0)
                        if tasks and tasks[0][0] <= it + LOOKAHEAD:
                            _, kind, fm, fn0 = tasks.pop(0)
                            run_task(kind, fm, fn0)
                        stt = stps.tile([128, sh], FP, name="stp", tag="stp")
                        for c in range(0, sh, 512):
                            nc.tensor.matmul(
                                stt[:, c : c + 512],
                                kt[po : po + 64, mt, m * 128 : (m + 1) * 128],
                                qt[po : po + 64, mt, c : c + 512],
                                start=True,
                                stop=True,
                            )
                        if m % 2 == 0:
                            p_pair = psb.tile([128, 2, sh], E4, name="pexp")
                        if m in (3, 8, 13):
                            fei = rpool.tile(
                                [128, sh], mybir.dt.int32, name="fei", bufs=3
                            )
                            nc.vector.tensor_scalar(
                                out=fei,
                                in0=stt,
                                scalar1=FE_A * scale / 256.0,
                                scalar2=fe_bcol[:, m : m + 1],
                                op0=OP.mult,
                                op1=OP.add,
                            )
                            nc.vector.tensor_copy(
                                out=p_pair[:, m % 2, :], in_=fei[:, :].bitcast(FP)
                            )
                        else:
                            nc.scalar.activation(
                                p_pair[:, m % 2, :],
                                stt,
                                AF.Exp,
                                bias=mask_sb[:, m : m + 1],
                                scale=scale / 256.0,
                            )
                        if m % 2 == 1:
                            for c in range(0, sh, 512):
                                nc.tensor.matmul(
                                    pv[:, c : c + 512],
                                    vsb[:, m - 1 : m + 1, hh * 65 : (hh + 1) * 65],
                                    p_pair[:, 0:2, c : c + 512],
                                    start=(m == 1),
                                    stop=(m == tt_n - 1),
                                    perf_mode=DR,
                                )
                        it += 1
                    # quick-free eviction: one DVE copy releases the single PV
                    # bank; the sum-row DRAM-roundtrip broadcast, reciprocal and
                    # normalize all run from the SBUF copy, off the PE path.
                    pvc = rpool.tile([65, sh], FP, name="pvc", bufs=3)
                    nc.vector.tensor_copy(out=pvc, in_=pv)
                    r_dram = dram.tile([sh], FP, name="rdram", tag="rdram", bufs=3)
                    nc.sync.dma_start(out=r_dram, in_=pvc[64:65, :])
                    # reciprocal cost scales with free size only: fold the 1024
                    # sums to [128, 8] for the reciprocal, then roundtrip again
                    # to broadcast the results across 64 partitions.
                    rbt = rpool.tile([128, sh // 128], FP, name="rbt", bufs=3)
                    nc.sync.dma_start(
                        out=rbt, in_=r_dram[:].rearrange("(p t) -> p t", p=128)
                    )
                    rit = rpool.tile([128, sh // 128], FP, name="rit", bufs=3)
                    nc.vector.reciprocal(rit, rbt)
                    r2_dram = dram.tile([sh], FP, name="r2dram", tag="r2dram", bufs=3)
                    nc.sync.dma_start(
                        out=r2_dram[:].rearrange("(p t) -> p t", p=128), in_=rit
                    )
                    rinv = rpool.tile([64, sh], FP, name="rinv", bufs=3)
                    nc.sync.dma_start(
                        out=rinv,
                        in_=bass.AP(
                            tensor=r2_dram.tensor,
                            offset=r2_dram.offset,
                            ap=[[0, 64], [1, sh]],
                        ),
                    )
                    nc.vector.tensor_mul(
                        out=ctx_t[po : po + 64, mt, :],
                        in0=pvc[0:64, :],
                        in1=rinv,
                    )
                for _, kind, fm, fn0 in tasks:  # leftovers (shouldn't happen)
                    run_task(kind, fm, fn0)
            wo_bf = wo_tiles[0]

            # ---- output projection + residual + LayerNorm (natural layout) ----
            with (
                tc.tile_pool(name="ops", bufs=8, space="PSUM") as ops,
                tc.tile_pool(name="xrp", bufs=qt_n) as xrp,
                tc.tile_pool(name="osb", bufs=3) as osb,
                tc.tile_pool(name="lnp", bufs=4) as lnp,
            ):
                xres_tiles = []
                for m in range(qt_n):
                    xr = xrp.tile([128, h], FP, name="xres", tag="xres")
                    nc.sync.dma_start(out=xr, in_=x[m * 128 : (m + 1) * 128, :])
                    xres_tiles.append(xr)
                for m in range(qt_n):
                    pss = []
                    for n0 in range(0, h, 512):
                        ps = ops.tile([128, 512], FP, name="op")
                        # ctx_t tile mt holds heads 2mt / 2mt+1 on partitions
                        # 0-63 / 64-127, exactly matching Wo rows mt*128..(mt+1)*128,
                        # so one K=128 matmul contracts both heads at once.
                        for mt in range(0, nh // 2, 2):
                            nc.tensor.matmul(
                                ps,
                                ctx_t[:, mt : mt + 2, m * 128 : (m + 1) * 128],
                                wo_bf[:, mt : mt + 2, n0 : n0 + 512],
                                start=(mt == 0),
                                stop=(mt == nh // 2 - 2),
                                perf_mode=DR,
                            )
                        pss.append((n0, ps))
                    xres = xres_tiles[m]
                    o = osb.tile([128, h], FP, name="osum")
                    for n0, ps in pss:
                        nc.vector.tensor_add(
                            out=o[:, n0 : n0 + 512], in0=ps, in1=xres[:, n0 : n0 + 512]
                        )
                    if "bo" in bcast:
                        nc.vector.tensor_add(out=o, in0=o, in1=bcast["bo"])
                    nsub = (h + 511) // 512
                    stats = lnp.tile([128, nsub, 6], FP, name="stats")
                    for i in range(nsub):
                        nc.vector.bn_stats(
                            out=stats[:, i, :], in_=o[:, i * 512 : (i + 1) * 512]
                        )
                    mv = lnp.tile([128, 2], FP, name="mv")
                    nc.vector.bn_aggr(out=mv, in_=stats)
                    std = lnp.tile([128, 1], FP, name="std")
                    nc.scalar.activation(std, mv[:, 1:2], AF.Sqrt, bias=eps_sb)
                    inv = lnp.tile([128, 1], FP, name="inv")
                    nc.vector.reciprocal(inv, std)
                    nb = lnp.tile([128, 1], FP, name="nb")
                    nc.vector.tensor_tensor(
                        out=nb, in0=mv[:, 0:1], in1=inv, op=OP.mult
                    )
                    nc.vector.tensor_scalar_mul(out=nb, in0=nb, scalar1=-1.0)
                    y = osb.tile([128, h], FP, name="yout")
                    nc.scalar.activation(
                        out=y, in_=o, func=AF.Identity, bias=nb, scale=inv
                    )
                    if "ln_gamma" in bcast:
                        nc.vector.tensor_mul(out=y, in0=y, in1=bcast["ln_gamma"])
                    if "ln_beta" in bcast:
                        nc.vector.tensor_add(out=y, in0=y, in1=bcast["ln_beta"])
                    nc.sync.dma_start(out=out[m * 128 : (m + 1) * 128, :], in_=y)

    _split_sync_waits(nc)
    return nc


_NC_CACHE = {}


def _get_nc(s, h, nh, sh, flags):
    key = (s, h, nh, sh, tuple(sorted(flags.items())))
    if key not in _NC_CACHE:
        _NC_CACHE[key] = _build(s, h, nh, sh, flags)
    return _NC_CACHE[key]


def _prepare(hidden_states, attention_mask, Wq, bq, Wk, bk, Wv, bv, Wo, bo, ln_gamma, ln_beta):
    hs = np.ascontiguousarray(np.asarray(hidden_states, dtype=np.float32))
    b_, s_, h_ = hs.shape
    nh_ = h_ // 64
    sh_ = s_ // 2
    am = np.asarray(attention_mask, dtype=np.float32).reshape(b_, s_)
    flags = {
        "bq": bool(np.any(np.asarray(bq))),
        "bk": bool(np.any(np.asarray(bk))),
        "bv": bool(np.any(np.asarray(bv))),
        "bo": bool(np.any(np.asarray(bo))),
        "ln_gamma": not bool(np.all(np.asarray(ln_gamma) == 1.0)),
        "ln_beta": bool(np.any(np.asarray(ln_beta))),
    }
    nc = _get_nc(s_, h_, nh_, sh_, flags)

    f32c = lambda a: np.ascontiguousarray(np.asarray(a, dtype=np.float32))
    f8c = lambda a, sc: np.ascontiguousarray(
        (np.asarray(a, dtype=np.float32) * sc).astype(ml_dtypes.float8_e4m3fn)
    )
    # weights x16 in fp8 (keeps small values out of the subnormal range);
    # K/Q both carry x16 so scores carry x256, folded into the Exp scale.
    # ctx_t carries x64 (x16 from V, x4 from the sum eviction), Wo x16, so
    # the out-proj PSUM carries x1024 — matched by scaling the residual
    # x1024 on the host. LayerNorm is scale-invariant, so the output is
    # unchanged.
    shared = {
        "wq": f8c(Wq, 16.0),
        "wk": f8c(Wk, 16.0),
        "wv": f8c(Wv, 64.0),
        "wo": f8c(Wo, 16.0),
    }
    scales = {"bq": 16.0, "bk": 16.0, "bv": 64.0, "bo": 1024.0}
    for name, arr in (
        ("bq", bq),
        ("bk", bk),
        ("bv", bv),
        ("bo", bo),
        ("ln_gamma", ln_gamma),
        ("ln_beta", ln_beta),
    ):
        if flags[name]:
            shared[name] = f32c(np.asarray(arr) * scales.get(name, 1.0))

    in_maps = []
    for c in range(N_CORES):
        bb, half = c // 2, c % 2
        mine = slice(half * sh_, (half + 1) * sh_)
        other = slice((1 - half) * sh_, (2 - half) * sh_)
        xp = np.concatenate([hs[bb, mine], hs[bb, other]], axis=0)
        xt = np.ascontiguousarray(xp.T.astype(ml_dtypes.float8_e4m3fn))
        mp = np.ascontiguousarray(np.concatenate([am[bb, mine], am[bb, other]]))
        in_maps.append(
            {
                "x": np.ascontiguousarray(xp[:sh_] * 1024.0),
                "xt": xt,
                "mask": mp,
                **shared,
            }
        )
    return nc, in_maps, (b_, s_, h_, sh_)


def _assemble(results, shape):
    b_, s_, h_, sh_ = shape
    out = np.empty((b_, s_, h_), dtype=np.float32)
    for c in range(N_CORES):
        bb, half = c // 2, c % 2
        out[bb, half * sh_ : (half + 1) * sh_] = results[c]["out"]
    return out


def kernel(**inputs) -> np.ndarray:
    nc, in_maps, shape = _prepare(**inputs)
    res = run_bass_kernel_spmd(nc, in_maps, core_ids=list(range(N_CORES)))
    return _assemble(res.results, shape)


# revision 29
# speedup vs baseline: 1.1619x; 1.0131x over previous
"""BertAttention (QKV proj + MHA + output proj + residual + LayerNorm) on 8 TRN2 NeuronCores.

Sharding: batch (4-way) x query-sequence-half (2-way) => 8 shards, no collectives.
Core c handles batch b=c//2, query half c%2. Each core computes K/V for its full
batch sequence (all heads) and Q/attention/output-proj/LayerNorm for its 1024
query rows. K/V projection work is duplicated across the 2 cores sharing a batch;
in exchange there is zero cross-core communication.

The host permutes each core's X rows so its query half comes first — attention is
permutation-invariant over keys as long as (K, V, mask) share the permutation, so
the program is identical across cores (pure SPMD) with no per-core indices.

Host pre-stages inputs: X transposed to [H, S] bf16 (feature on partitions after
DMA), weights cast to bf16, residual rows kept fp32. This removes all on-device
casts and PE transposes and halves the load DMA bytes.

Layouts (SBUF partition dim first):
  xt:      [128, H/128, S]   transposed activations, bf16 (direct DMA)
  Kt:      [128, H/128, S]   transposed keys (feature on partitions), bf16
  Qt:      [128, H/128, SH]  transposed, bf16
  V:       [128, S/128, NH*65] natural ([tok, head-dim]) with a ones column per
           head at slot 64 — the PV matmul then yields sum(exp) as row 64 for free
  scores:  St[ktok, qtok] in PSUM; softmax sum over ktok (the partition dim) comes
           from the ones-column trick; max-subtraction safely skipped (|s| <~ 1)
  ctx:     [128, NH/2, SH]   transposed (head dim on partitions), bf16
  out:     natural [qtok, H] — residual add + LayerNorm along the free dim.

Schedule: K/Q tile 0 + V chunk 0 projected upfront (~25us, overlapped with the
weight DMA), then the 16-head attention loop starts; remaining projection groups
are deadline-paced into the loop (one group per scores iteration) as PE
gap-filler. PV PSUM accumulators ping-pong across heads so the softmax-sum
eviction (DMA roundtrip broadcast + reciprocal + multiply) never stalls the PE.
"""

from contextlib import ExitStack

import numpy as np
import ml_dtypes

import bass_rust
import concourse.bass as bass
import concourse.mybir as mybir
from concourse.tile import TileContext
from concourse.bass_utils import run_bass_kernel_spmd

FP = mybir.dt.float32
BF = mybir.dt.bfloat16
E4 = mybir.dt.float8e4
DR = mybir.MatmulPerfMode.DoubleRow
AF = mybir.ActivationFunctionType
OP = mybir.AluOpType

N_CORES = 8
EPS = 1e-12

# The walrus build in this toolchain rejects instructions that carry more than
# one sync-wait command ("Too many sync wait commands", CoreV2/V3 setupSyncWait),
# while Tile freely attaches several semaphore waits to one instruction (and the
# TileContext exit drain aggregates one wait per logical processor). Hoist the
# excess waits onto standalone InstEventSemaphore carriers on the same engine,
# placed immediately before the instruction — engine streams are serial, so the
# gating semantics are identical.
_MAX_WAITS_PER_INST = 1


def _split_sync_waits(nc, cap=_MAX_WAITS_PER_INST):
    n_split = 0
    for fn in nc.m.functions:
        for bb in fn.blocks:
            insts = list(bb.instructions)
            out = []
            changed = False
            for ins in insts:
                si = ins.sync_info
                waits = list(si.on_wait) if (si is not None and si.on_wait) else []
                if len(waits) > cap:
                    head, tail = waits[: len(waits) - cap], waits[len(waits) - cap :]
                    for j, w in enumerate(head):
                        ev = mybir.InstEventSemaphore(
                            name=f"{ins.name}-sw{j}",
                            engine=ins.engine,
                            ins=[],
                            outs=[],
                            sync_info=bass_rust.SyncInfo(on_wait=[w], on_update=[]),
                        )
                        out.append(ev)
                        n_split += 1
                    si.on_wait = tail
                    changed = True
                out.append(ins)
            if changed:
                bb.instructions[:] = out
    return n_split


def _dram_row_bcast(handle, p, n):
    """AP reading DRAM vector [n] broadcast across p partitions."""
    return bass.AP(tensor=handle, offset=0, ap=[[0, p], [1, n]])


def _build(s, h, nh, sh, flags):
    """Build the per-core Bass program. flags: which bias/affine inputs matter."""
    hd = h // nh
    assert hd == 64, "head packing assumes head_dim 64 (2 heads per 128 partitions)"
    kt_n = h // 128  # contraction tiles over hidden dim
    tt_n = s // 128  # key-token tiles
    qt_n = sh // 128  # query-token tiles
    scale = 1.0 / float(np.sqrt(hd))

    nc = bass.Bass(target_bir_lowering=False)
    x = nc.dram_tensor("x", [sh, h], FP, kind="ExternalInput")  # residual rows
    xt_d = nc.dram_tensor("xt", [h, s], E4, kind="ExternalInput")  # transposed
    mask = nc.dram_tensor("mask", [s], FP, kind="ExternalInput")
    w_dram = {
        n: nc.dram_tensor(n, [h, h], E4, kind="ExternalInput")
        for n in ("wq", "wk", "wv", "wo")
    }
    vec_dram = {
        n: nc.dram_tensor(n, [h], FP, kind="ExternalInput")
        for n in ("bq", "bk", "bv", "bo", "ln_gamma", "ln_beta")
        if flags[n]
    }
    out = nc.dram_tensor("out", [sh, h], FP, kind="ExternalOutput")

    with TileContext(nc) as tc, ExitStack() as st_all:
        persist = st_all.enter_context(tc.tile_pool(name="persist", bufs=1))
        dram = st_all.enter_context(tc.tile_pool(name="dram", bufs=1, space="DRAM"))
        qt = persist.tile([128, kt_n, sh], BF)
        kt = persist.tile([128, kt_n, s], BF)
        vsb = persist.tile([128, tt_n, nh * 65], E4)
        ctx_t = persist.tile([128, nh // 2, sh], E4)
        mask_sb = persist.tile([128, tt_n], FP)
        eps_sb = persist.tile([128, 1], FP)

        nc.vector.memset(eps_sb, EPS)
        nc.sync.dma_start(out=mask_sb, in_=mask[:].rearrange("(t p) -> p t", p=128))

        # Schraudolph fast-exp (exp(y) ~ bitcast_f32(i32(y*A + B))) constants;
        # a few exp tiles per head run as DVE int-convert + GpSimd bitcast-copy
        # to offload the Scalar engine (max ~4% per-element error, harmless
        # after softmax normalization and the residual).
        FE_A = 12102203.161561485
        FE_B = 1064866805.0
        fe_bcol = persist.tile([128, tt_n], FP, name="fe_bcol")
        nc.vector.tensor_scalar(
            out=fe_bcol,
            in0=mask_sb,
            scalar1=FE_A,
            scalar2=FE_B,
            op0=OP.mult,
            op1=OP.add,
        )

        # bias columns for Qt/Kt evictions (partition = output feature in tile)
        bias_cols = {}
        for name in ("bq", "bk"):
            if flags[name]:
                col = persist.tile([128, kt_n], FP, name=f"{name}_col")
                nc.sync.dma_start(
                    out=col, in_=vec_dram[name][:].rearrange("(t p) -> p t", p=128)
                )
                bias_cols[name] = col
        # rows broadcast across partitions for V/out bias and LN affine
        bcast = {}
        for name in ("bv", "bo", "ln_gamma", "ln_beta"):
            if flags[name]:
                t = persist.tile([128, h], FP, name=f"{name}_bc")
                nc.sync.dma_start(out=t, in_=_dram_row_bcast(vec_dram[name], 128, h))
                bcast[name] = t

        # ones columns in V (slot 64 of each 65-wide head block)
        for m in range(tt_n):
            v_view = vsb[:, m, :].rearrange("p (a e) -> p a e", e=65)
            nc.vector.memset(v_view[:, :, 64:65], 1.0)

        with ExitStack() as st_proj:
            xtpool = st_proj.enter_context(tc.tile_pool(name="xtpool", bufs=1))
            wbuf = st_proj.enter_context(tc.tile_pool(name="wbuf", bufs=3))

            xt = xtpool.tile([128, kt_n, s], E4)

            # slot assignment: wv takes slot 0 so the late wo load (issued
            # after the last V fill group) can reuse it; wk/wq live through
            # the whole attention loop (K/Q fill groups read them).
            wv_bf = wbuf.tile([128, kt_n, h], E4, name="wv_bf", tag="w")
            wk_bf = wbuf.tile([128, kt_n, h], E4, name="wk_bf", tag="w")
            wq_bf = wbuf.tile([128, kt_n, h], E4, name="wq_bf", tag="w")

            def load_w(dname, dst):
                for k in range(kt_n):
                    nc.sync.dma_start(
                        out=dst[:, k, :], in_=w_dram[dname][k * 128 : (k + 1) * 128, :]
                    )

            # DMA issue order = priority: xt + wk first (unblocks K tile 0),
            # then wq, then wv. wo is issued much later.
            for k in range(kt_n):
                nc.sync.dma_start(
                    out=xt[:, k, :], in_=xt_d[k * 128 : (k + 1) * 128, :]
                )
            load_w("wk", wk_bf)
            load_w("wq", wq_bf)
            load_w("wv", wv_bf)

            def kq_group(w_bf, dst, bias_col, m, n0, pool, pool_shape, tag):
                """One K/Q projection PSUM group: 8 accumulating matmuls + evict."""
                n1 = min(n0 + 512, dst.shape[2])
                ps = pool.tile(pool_shape, FP, name="projp", tag=tag)
                for k in range(0, kt_n, 2):
                    nc.tensor.matmul(
                        ps[:, : n1 - n0],
                        w_bf[:, k : k + 2, m * 128 : (m + 1) * 128],
                        xt[:, k : k + 2, n0:n1],
                        start=(k == 0),
                        stop=(k == kt_n - 2),
                        perf_mode=DR,
                    )
                if bias_col is not None:
                    nc.vector.tensor_scalar_add(
                        out=dst[:, m, n0:n1],
                        in0=ps[:, : n1 - n0],
                        scalar1=bias_col[:, m : m + 1],
                    )
                else:
                    nc.vector.tensor_copy(out=dst[:, m, n0:n1], in_=ps[:, : n1 - n0])

            def v_group(m, n0, pool, pool_shape, tag):
                ps = pool.tile(pool_shape, FP, name="projp", tag=tag)
                for k in range(0, kt_n, 2):
                    nc.tensor.matmul(
                        ps[:, :512],
                        xt[:, k : k + 2, m * 128 : (m + 1) * 128],
                        wv_bf[:, k : k + 2, n0 : n0 + 512],
                        start=(k == 0),
                        stop=(k == kt_n - 2),
                        perf_mode=DR,
                    )
                dst = vsb[:, m, :].rearrange("p (a e) -> p a e", e=65)[
                    :, n0 // 64 : n0 // 64 + 8, 0:64
                ]
                src = ps[:, :512].rearrange("p (a e) -> p a e", e=64)
                if "bv" in bcast:
                    nc.vector.tensor_add(
                        out=dst,
                        in0=src,
                        in1=bcast["bv"][:, n0 : n0 + 512].rearrange(
                            "p (a e) -> p a e", e=64
                        ),
                    )
                else:
                    nc.vector.tensor_copy(out=dst, in_=src)

            # ---- upfront projections (overlap the weight DMA) ----
            with tc.tile_pool(name="projps", bufs=2, space="PSUM") as projps:
                for n0 in range(0, s, 512):
                    kq_group(
                        wk_bf, kt, bias_cols.get("bk"), 0, n0, projps, [128, 512], "projp"
                    )
                for n0 in range(0, sh, 512):
                    kq_group(
                        wq_bf, qt, bias_cols.get("bq"), 0, n0, projps, [128, 512], "projp"
                    )
                for m in range(tt_n):
                    v_group(m, 0, projps, [128, 512], "projp")

            # fill tasks, deadline in scores-iteration units (16 per head)
            tasks = []
            for m in range(1, kt_n):
                for n0 in range(0, s, 512):
                    tasks.append((32 * m, "k", m, n0))
                for n0 in range(0, sh, 512):
                    tasks.append((32 * m, "q", m, n0))
            for m in range(tt_n):
                tasks.append((8 * tt_n + m, "v", m, 512))
            tasks.sort()
            n_it_total = nh * tt_n
            tasks = [
                (min(dl - 8, round((i + 0.5) * n_it_total / len(tasks))), kind, fm, fn0)
                for i, (dl, kind, fm, fn0) in enumerate(tasks)
            ]
            tasks.sort()
            wo_issued = False
            n_v_left = tt_n

            # ---- attention, with projection fill interleaved ----
            with ExitStack() as st_att:
                psb = st_att.enter_context(tc.tile_pool(name="psb", bufs=3))
                rpool = st_att.enter_context(tc.tile_pool(name="rpool", bufs=2))
                stps = st_att.enter_context(
                    tc.tile_pool(name="stps", bufs=3, space="PSUM")
                )
                pvps = st_att.enter_context(
                    tc.tile_pool(name="pvps", bufs=1, space="PSUM")
                )
                LOOKAHEAD = 24

                def run_task(kind, fm, fn0):
                    nonlocal n_v_left, wo_issued
                    if kind == "k":
                        kq_group(
                            wk_bf, kt, bias_cols.get("bk"), fm, fn0, stps, [128, sh], "stp"
                        )
                    elif kind == "q":
                        kq_group(
                            wq_bf, qt, bias_cols.get("bq"), fm, fn0, stps, [128, sh], "stp"
                        )
                    else:
                        v_group(fm, fn0, stps, [128, sh], "stp")
                        n_v_left -= 1
                        if n_v_left == 0 and not wo_issued:
                            wo_issued = True
                            wo_tiles.append(wbuf.tile([128, kt_n, h], E4, name="wo_bf", tag="w"))
                            load_w("wo", wo_tiles[0])

                wo_tiles = []
                it = 0
                for hh in range(nh):
                    mt, po = hh // 2, 64 * (hh % 2)
                    pv = pvps.tile([65, sh], FP, name="pvp")
                    for m in range(tt_n):
                        stt = stps.tile([128, sh], FP, name="stp", tag="stp")
                        for c in range(0, sh, 512):
                            nc.tensor.matmul(
                                stt[:, c : c + 512],
                                kt[po : po + 64, mt, m * 128 : (m + 1) * 128],
                                qt[po : po + 64, mt, c : c + 512],
                                start=True,
                                stop=True,
                            )
                        if m % 2 == 0:
                            p_pair = psb.tile([128, 2, sh], E4, name="pexp")
                        if m in (3, 8, 13):
                            fei = rpool.tile(
                                [128, sh], mybir.dt.int32, name="fei", bufs=3
                            )
                            nc.vector.tensor_scalar(
                                out=fei,
                                in0=stt,
                                scalar1=FE_A * scale / 256.0,
                                scalar2=fe_bcol[:, m : m + 1],
                                op0=OP.mult,
                                op1=OP.add,
                            )
                            nc.vector.tensor_copy(
                                out=p_pair[:, m % 2, :], in_=fei[:, :].bitcast(FP)
                            )
                        else:
                            nc.scalar.activation(
                                p_pair[:, m % 2, :],
                                stt,
                                AF.Exp,
                                bias=mask_sb[:, m : m + 1],
                                scale=scale / 256.0,
                            )
                        if m % 2 == 1:
                            for c in range(0, sh, 512):
                                nc.tensor.matmul(
                                    pv[:, c : c + 512],
                                    vsb[:, m - 1 : m + 1, hh * 65 : (hh + 1) * 65],
                                    p_pair[:, 0:2, c : c + 512],
                                    start=(m == 1),
                                    stop=(m == tt_n - 1),
                                    perf_mode=DR,
                                )
                        while tasks and tasks[0][0] <= it:
                            _, kind, fm, fn0 = tasks.pop(0)
                            run_task(kind, fm, fn0)
                        if tasks and tasks[0][0] <= it + LOOKAHEAD:
                            _, kind, fm, fn0 = tasks.pop(0)
                            run_task(kind, fm, fn0)
                        it += 1
                    # quick-free eviction: one DVE copy releases the single PV
                    # bank; the sum-row DRAM-roundtrip broadcast, reciprocal and
                    # normalize all run from the SBUF copy, off the PE path.
                    pvc = rpool.tile([65, sh], FP, name="pvc", bufs=3)
                    nc.vector.tensor_copy(out=pvc, in_=pv)
                    r_dram = dram.tile([sh], FP, name="rdram", tag="rdram", bufs=3)
                    nc.sync.dma_start(out=r_dram, in_=pvc[64:65, :])
                    # reciprocal cost scales with free size only: fold the 1024
                    # sums to [128, 8] for the reciprocal, then roundtrip again
                    # to broadcast the results across 64 partitions.
                    rbt = rpool.tile([128, sh // 128], FP, name="rbt", bufs=3)
                    nc.sync.dma_start(
                        out=rbt, in_=r_dram[:].rearrange("(p t) -> p t", p=128)
                    )
                    rit = rpool.tile([128, sh // 128], FP, name="rit", bufs=3)
                    nc.vector.reciprocal(rit, rbt)
                    r2_dram = dram.tile([sh], FP, name="r2dram", tag="r2dram", bufs=3)
                    nc.sync.dma_start(
                        out=r2_dram[:].rearrange("(p t) -> p t", p=128), in_=rit
                    )
                    rinv = rpool.tile([64, sh], FP, name="rinv", bufs=3)
                    nc.sync.dma_start(
                        out=rinv,
                        in_=bass.AP(
                            tensor=r2_dram.tensor,
                            offset=r2_dram.offset,
                            ap=[[0, 64], [1, sh]],
                        ),
                    )
                    nc.vector.tensor_mul(
                        out=ctx_t[po : po + 64, mt, :],
                        in0=pvc[0:64, :],
                        in1=rinv,
                    )
                for _, kind, fm, fn0 in tasks:  # leftovers (shouldn't happen)
                    run_task(kind, fm, fn0)
            wo_bf = wo_tiles[0]

            # ---- output projection + residual + LayerNorm (natural layout) ----
            with (
                tc.tile_pool(name="ops", bufs=8, space="PSUM") as ops,
                tc.tile_pool(name="xrp", bufs=qt_n) as xrp,
                tc.tile_pool(name="osb", bufs=3) as osb,
                tc.tile_pool(name="lnp", bufs=4) as lnp,
            ):
                xres_tiles = []
                for m in range(qt_n):
                    xr = xrp.tile([128, h], FP, name="xres", tag="xres")
                    nc.sync.dma_start(out=xr, in_=x[m * 128 : (m + 1) * 128, :])
                    xres_tiles.append(xr)
                for m in range(qt_n):
                    pss = []
                    for n0 in range(0, h, 512):
                        ps = ops.tile([128, 512], FP, name="op")
                        # ctx_t tile mt holds heads 2mt / 2mt+1 on partitions
                        # 0-63 / 64-127, exactly matching Wo rows mt*128..(mt+1)*128,
                        # so one K=128 matmul contracts both heads at once.
                        for mt in range(0, nh // 2, 2):
                            nc.tensor.matmul(
                                ps,
                                ctx_t[:, mt : mt + 2, m * 128 : (m + 1) * 128],
                                wo_bf[:, mt : mt + 2, n0 : n0 + 512],
                                start=(mt == 0),
                                stop=(mt == nh // 2 - 2),
                                perf_mode=DR,
                            )
                        pss.append((n0, ps))
                    xres = xres_tiles[m]
                    o = osb.tile([128, h], FP, name="osum")
                    for n0, ps in pss:
                        nc.vector.tensor_add(
                            out=o[:, n0 : n0 + 512], in0=ps, in1=xres[:, n0 : n0 + 512]
                        )
                    if "bo" in bcast:
                        nc.vector.tensor_add(out=o, in0=o, in1=bcast["bo"])
                    nsub = (h + 511) // 512
                    stats = lnp.tile([128, nsub, 6], FP, name="stats")
                    for i in range(nsub):
                        nc.vector.bn_stats(
                            out=stats[:, i, :], in_=o[:, i * 512 : (i + 1) * 512]
                        )
                    mv = lnp.tile([128, 2], FP, name="mv")
                    nc.vector.bn_aggr(out=mv, in_=stats)
                    std = lnp.tile([128, 1], FP, name="std")
                    nc.scalar.activation(std, mv[:, 1:2], AF.Sqrt, bias=eps_sb)
                    inv = lnp.tile([128, 1], FP, name="inv")
                    nc.vector.reciprocal(inv, std)
                    nb = lnp.tile([128, 1], FP, name="nb")
                    nc.vector.tensor_tensor(
                        out=nb, in0=mv[:, 0:1], in1=inv, op=OP.mult
                    )
                    nc.vector.tensor_scalar_mul(out=nb, in0=nb, scalar1=-1.0)
                    y = osb.tile([128, h], FP, name="yout")
                    nc.scalar.activation(
                        out=y, in_=o, func=AF.Identity, bias=nb, scale=inv
                    )
                    if "ln_gamma" in bcast:
                        nc.vector.tensor_mul(out=y, in0=y, in1=bcast["ln_gamma"])
                    if "ln_beta" in bcast:
                        nc.vector.tensor_add(out=y, in0=y, in1=bcast["ln_beta"])
                    nc.sync.dma_start(out=out[m * 128 : (m + 1) * 128, :], in_=y)

    _split_sync_waits(nc)
    return nc


_NC_CACHE = {}


def _get_nc(s, h, nh, sh, flags):
    key = (s, h, nh, sh, tuple(sorted(flags.items())))
    if key not in _NC_CACHE:
        _NC_CACHE[key] = _build(s, h, nh, sh, flags)
    return _NC_CACHE[key]


def _prepare(hidden_states, attention_mask, Wq, bq, Wk, bk, Wv, bv, Wo, bo, ln_gamma, ln_beta):
    hs = np.ascontiguousarray(np.asarray(hidden_states, dtype=np.float32))
    b_, s_, h_ = hs.shape
    nh_ = h_ // 64
    sh_ = s_ // 2
    am = np.asarray(attention_mask, dtype=np.float32).reshape(b_, s_)
    flags = {
        "bq": bool(np.any(np.asarray(bq))),
        "bk": bool(np.any(np.asarray(bk))),
        "bv": bool(np.any(np.asarray(bv))),
        "bo": bool(np.any(np.asarray(bo))),
        "ln_gamma": not bool(np.all(np.asarray(ln_gamma) == 1.0)),
        "ln_beta": bool(np.any(np.asarray(ln_beta))),
    }
    nc = _get_nc(s_, h_, nh_, sh_, flags)

    f32c = lambda a: np.ascontiguousarray(np.asarray(a, dtype=np.float32))
    f8c = lambda a, sc: np.ascontiguousarray(
        (np.asarray(a, dtype=np.float32) * sc).astype(ml_dtypes.float8_e4m3fn)
    )
    # weights x16 in fp8 (keeps small values out of the subnormal range);
    # K/Q both carry x16 so scores carry x256, folded into the Exp scale.
    # ctx_t carries x64 (x16 from V, x4 from the sum eviction), Wo x16, so
    # the out-proj PSUM carries x1024 — matched by scaling the residual
    # x1024 on the host. LayerNorm is scale-invariant, so the output is
    # unchanged.
    shared = {
        "wq": f8c(Wq, 16.0),
        "wk": f8c(Wk, 16.0),
        "wv": f8c(Wv, 64.0),
        "wo": f8c(Wo, 16.0),
    }
    scales = {"bq": 16.0, "bk": 16.0, "bv": 64.0, "bo": 1024.0}
    for name, arr in (
        ("bq", bq),
        ("bk", bk),
        ("bv", bv),
        ("bo", bo),
        ("ln_gamma", ln_gamma),
        ("ln_beta", ln_beta),
    ):
        if flags[name]:
            shared[name] = f32c(np.asarray(arr) * scales.get(name, 1.0))

    in_maps = []
    for c in range(N_CORES):
        bb, half = c // 2, c % 2
        mine = slice(half * sh_, (half + 1) * sh_)
        other = slice((1 - half) * sh_, (2 - half) * sh_)
        xp = np.concatenate([hs[bb, mine], hs[bb, other]], axis=0)
        xt = np.ascontiguousarray(xp.T.astype(ml_dtypes.float8_e4m3fn))
        mp = np.ascontiguousarray(np.concatenate([am[bb, mine], am[bb, other]]))
        in_maps.append(
            {
                "x": np.ascontiguousarray(xp[:sh_] * 1024.0),
                "xt": xt,
                "mask": mp,
                **shared,
            }
        )
    return nc, in_maps, (b_, s_, h_, sh_)


def _assemble(results, shape):
    b_, s_, h_, sh_ = shape
    out = np.empty((b_, s_, h_), dtype=np.float32)
    for c in range(N_CORES):
        bb, half = c // 2, c % 2
        out[bb, half * sh_ : (half + 1) * sh_] = results[c]["out"]
    return out


def kernel(**inputs) -> np.ndarray:
    nc, in_maps, shape = _prepare(**inputs)
    res = run_bass_kernel_spmd(nc, in_maps, core_ids=list(range(N_CORES)))
    return _assemble(res.results, shape)
